# revision 1
# baseline (speedup 1.0000x reference)
"""MemoryReader kernel for Trainium2, data-parallel over batch across 8 cores.

Per batch element b (one NeuronCore each):
    mkf = mk[b] as [CK=64, M=4096], qkf = qk[b] as [CK, N=4096]
    aff[m, n] = (2 * mkf.T @ qkf - |mkf[:,m]|^2) / sqrt(CK)
    P = softmax over m
    mem[c, n]  = sum_m mv[b][c, m] * P[m, n]
    out[b] = concat([mem, qv[b]], channel axis)

Device kernel layout (per core):
    - QK^T matmuls produce aff tiles in [m-partition, n-free] layout,
      32 m-chunks of [128, 512] per n-super-tile of 512 columns.
    - ScalarE computes E = exp(0.25*ab - a_sq/8) straight out of PSUM
      (per-partition bias = -a_sq/8; logits are bounded so the max
      subtraction of a standard softmax is unnecessary in fp32).
    - VectorE accumulates sum_m E chunk-by-chunk; a ones-vector matmul
      folds the partition axis; reciprocal + DMA partition-broadcast
      give 1/s replicated across partitions.
    - Readout matmuls contract over m in PSUM (4 c-chunks of 128), then
      VectorE scales by 1/s while evacuating PSUM.
    - mv^T / mk^T are prepared host-side (pure layout transforms), so no
      on-device transposes are needed. qv never touches the device.
"""

import os
import sys

import numpy as np

B, CK, CV, H, W = 8, 64, 512, 64, 64
M = H * W          # memory positions per batch element
N = H * W          # query positions
NT = 512           # n-super-tile width (columns per softmax pass)
NSUP = N // NT     # 8 n-super-tiles
MCH = M // 128     # 32 m-chunks
N_CORES = 8

# "fp32r" runs matmuls in relaxed-precision single-pass mode (4x faster
# than exact fp32 on the PE array); "fp32" is exact.
MATMUL_PREC = os.environ.get("KERNEL_MATMUL_PREC", "fp32r")

_CACHE = {}


def _build_program():
    sys.path.insert(0, "/opt/trn_rl_repo")
    from contextlib import ExitStack

    import concourse.tile as tile
    from concourse import bacc, mybir

    dt = mybir.dt
    f32 = dt.float32
    # Matmul operand dtype: float32r (relaxed single-pass fp32, 4x faster
    # on the PE array) or exact float32. Bit-layout is identical; walrus
    # requires producers of fp32r matmul operands to be typed fp32r.
    mdt = dt.float32r if MATMUL_PREC == "fp32r" else f32

    nc = bacc.Bacc("TRN2", target_bir_lowering=False, debug=False,
                   num_devices=N_CORES)

    mk_d = nc.dram_tensor("mk", [128, M], mdt, kind="ExternalInput").ap()
    mkt_d = nc.dram_tensor("mkt", [128, MCH * CK], f32,
                           kind="ExternalInput").ap()
    qk_d = nc.dram_tensor("qk", [128, N], mdt, kind="ExternalInput").ap()
    mvt_d = nc.dram_tensor("mvt", [MCH, 128, CV], mdt,
                           kind="ExternalInput").ap()
    mem_d = nc.dram_tensor("mem", [CV, N], f32, kind="ExternalOutput").ap()

    with tile.TileContext(nc) as tc, ExitStack() as ctx:
        sing = ctx.enter_context(tc.tile_pool(name="sing", bufs=1))
        e_pool = ctx.enter_context(tc.tile_pool(name="E", bufs=17))
        scratch = ctx.enter_context(tc.tile_pool(name="scratch", bufs=2))
        sacc_pool = ctx.enter_context(tc.tile_pool(name="sacc", bufs=2))
        row_pool = ctx.enter_context(tc.tile_pool(name="row", bufs=2))
        rb_pool = ctx.enter_context(tc.tile_pool(name="rb", bufs=2))
        out_pool = ctx.enter_context(tc.tile_pool(name="out", bufs=8))
        qk_ps_pool = ctx.enter_context(
            tc.tile_pool(name="qkps", bufs=2, space="PSUM"))
        ro_ps_pool = ctx.enter_context(
            tc.tile_pool(name="rops", bufs=1, space="PSUM"))


        # PE warmup: the PE activity monitor starts throttled at 1.2 GHz
        # and needs ~3.4us of sustained matmul activity to unthrottle.
        # Burn dummy matmuls while the input DMAs stream so the real
        # matmuls start at 2.4 GHz.
        warm_sb = sing.tile([128, NT], f32)
        nc.vector.memset(warm_sb[:], 1.0)
        warm_ps = qk_ps_pool.tile([128, NT], f32, tag="qk_ps", name="warm_ps")
        for w in range(56):
            nc.tensor.matmul(warm_ps[:, 0:128], lhsT=warm_sb[:, 0:128],
                             rhs=warm_sb[:, 0:128], start=True, stop=True)

        # Resident inputs. mk/qk are zero-padded from CK=64 to K=128
        # contraction rows: K=64 matmuls leave the PE activity monitor
        # throttled at 1.2 GHz (measured 427 ns/MM vs 222 ns at K=128),
        # so padded K=128 matmuls are 2x faster despite wasting rows.
        # All DMAs go through the sync engine (hardware DGE); ordered so
        # the tensors gating the first matmuls arrive first.
        mk_sb = sing.tile([128, M], mdt)
        qk_sb = sing.tile([128, N], mdt)
        mkt_sb = sing.tile([128, MCH, CK], f32)
        mvt_sb = sing.tile([128, MCH, CV], mdt)
        for g in range(4):
            gs = slice(g * 1024, (g + 1) * 1024)
            nc.sync.dma_start(out=mk_sb[:, gs], in_=mk_d[:, gs])
        nc.sync.dma_start(out=qk_sb[:, 0:NT], in_=qk_d[:, 0:NT])
        nc.sync.dma_start(out=mkt_sb[:], in_=mkt_d[:].rearrange(
            "p (j c) -> p j c", c=CK))
        for j in range(4):
            nc.sync.dma_start(out=mvt_sb[:, j, :], in_=mvt_d[j])
        nc.sync.dma_start(out=qk_sb[:, NT:N], in_=qk_d[:, NT:N])
        for j in range(4, MCH):
            nc.sync.dma_start(out=mvt_sb[:, j, :], in_=mvt_d[j])

        # Ones vectors typed fp32r so the softmax-sum and broadcast
        # matmuls take the single-pass PE path (213 ns vs 853 ns).
        ones_f32 = sing.tile([128, 1], f32)
        nc.vector.memset(ones_f32[:], 1.0)
        ones_sb = sing.tile([128, 1], mdt)
        nc.vector.tensor_copy(ones_sb[:], ones_f32[:].bitcast(mdt))
        ones_row_f32 = sing.tile([1, 128], f32)
        nc.vector.memset(ones_row_f32[:], 1.0)
        ones_row = sing.tile([1, 128], mdt)
        nc.vector.tensor_copy(ones_row[:], ones_row_f32[:].bitcast(mdt))

        # Per-partition softmax bias: asq[p, j] = -|mk[:, j*128+p]|^2 / 8.
        # (tensor_tensor_reduce crashes on HW via this toolchain; use
        # Square -> free-axis reduce -> scale, in 4 pieces to keep the
        # scratch small.)
        asq = sing.tile([128, MCH], f32)
        for piece in range(4):
            js = slice(piece * 8, (piece + 1) * 8)
            sqp = scratch.tile([128, 8, CK], f32, tag="sqp",
                               name=f"sqp{piece}")
            nc.scalar.activation(sqp[:], mkt_sb[:, js, :],
                                 mybir.ActivationFunctionType.Square)
            nc.vector.tensor_reduce(asq[:, js], sqp[:],
                                    axis=mybir.AxisListType.X,
                                    op=mybir.AluOpType.add)
        nc.scalar.mul(asq[:], asq[:], -0.125)
        # g[p, j] = exp(-|mk row|^2 / 8); folded into the value rows and
        # the denominator accumulation so the exp needs no bias and can
        # span two PSUM banks per instruction.
        g_col = sing.tile([128, MCH], f32)
        nc.scalar.activation(g_col[:], asq[:],
                             mybir.ActivationFunctionType.Exp)
        with nc.allow_low_precision(reason="fp32r is fp32 bits"):
            for j in range(MCH):
                nc.vector.tensor_scalar_mul(mvt_sb[:, j, :],
                                            mvt_sb[:, j, :],
                                            g_col[:, j:j + 1])

        def emit_tail(ti, tsacc, tosbs, tnsl):
            # Softmax denominator, reciprocal, partition-broadcast and
            # final scaling for super `ti`. Emitted a few chunks into the
            # NEXT super so the PE stream has QK matmuls to chew on while
            # the DVE-side reduction chain resolves.
            s_ps = qk_ps_pool.tile([1, NT], f32, tag="qk_ps",
                                   name=f"sps{ti}")
            nc.tensor.matmul(s_ps[:], lhsT=ones_sb[:], rhs=tsacc[:],
                             start=True, stop=True)
            s_row = row_pool.tile([1, NT], mdt, tag="srow",
                                  name=f"srow{ti}")
            with nc.allow_low_precision(reason="fp32r is fp32 bits"):
                nc.vector.reciprocal(s_row[:], s_ps[:].bitcast(mdt))
            rb_ps = qk_ps_pool.tile([128, NT], f32, tag="qk_ps",
                                    name=f"rbps{ti}")
            nc.tensor.matmul(rb_ps[:], lhsT=ones_row[:], rhs=s_row[:],
                             start=True, stop=True)
            rb = rb_pool.tile([128, NT], f32, tag="rb", name=f"rb{ti}")
            nc.scalar.copy(rb[:], rb_ps[:])
            for c in range(4):
                nc.vector.tensor_mul(tosbs[c][:], tosbs[c][:], rb[:])
                nc.sync.dma_start(
                    out=mem_d[c * 128:(c + 1) * 128, tnsl], in_=tosbs[c][:])

        pending_tail = None
        for i in range(NSUP):
            nsl = slice(i * NT, (i + 1) * NT)
            ro_ps = [ro_ps_pool.tile([128, NT], f32, tag=f"ro{c}",
                                     name=f"ro{c}_{i}")
                     for c in range(4)]
            sacc = sacc_pool.tile([128, NT], mdt, tag="sacc",
                                  name=f"sacc{i}")
            for t in range(MCH // 2):
                ma, mb = 2 * t, 2 * t + 1
                qk_ps = qk_ps_pool.tile([128, 2 * NT], f32, tag="qk_ps",
                                        name=f"qkps{i}_{t}")
                for h, m in ((0, ma), (1, mb)):
                    nc.tensor.matmul(
                        qk_ps[:, h * NT:(h + 1) * NT],
                        lhsT=mk_sb[:, m * 128:(m + 1) * 128],
                        rhs=qk_sb[:, nsl],
                        start=True, stop=True)
                e = e_pool.tile([128, 2 * NT], mdt, tag="E",
                                name=f"e{i}_{t}")
                nc.scalar.activation(
                    e[:], qk_ps[:], mybir.ActivationFunctionType.Exp,
                    scale=0.25)
                # sacc += g[m] * E chunk; fp32r is bit-identical to fp32,
                # the low-precision gate only keys off the dtype tag.
                with nc.allow_low_precision(reason="fp32r is fp32 bits"):
                    for h, m in ((0, ma), (1, mb)):
                        eh = e[:, h * NT:(h + 1) * NT]
                        if m == 0:
                            nc.vector.tensor_scalar_mul(
                                sacc[:], eh, g_col[:, m:m + 1])
                        else:
                            nc.vector.scalar_tensor_tensor(
                                out=sacc[:], in0=eh,
                                scalar=g_col[:, m:m + 1], in1=sacc[:],
                                op0=mybir.AluOpType.mult,
                                op1=mybir.AluOpType.add)
                if t == 2 and pending_tail is not None:
                    emit_tail(*pending_tail)
                    pending_tail = None
                for h, m in ((0, ma), (1, mb)):
                    for c in range(4):
                        nc.tensor.matmul(
                            ro_ps[c][:],
                            lhsT=mvt_sb[:, m, c * 128:(c + 1) * 128],
                            rhs=e[:, h * NT:(h + 1) * NT],
                            start=(m == 0), stop=(m == MCH - 1))

            # Evacuate readout PSUM unscaled right away so the next
            # n-super's readout matmuls get their banks back without
            # waiting on the softmax-sum/reciprocal chain.
            osbs = []
            for c in range(4):
                osb = out_pool.tile([128, NT], f32, tag="osb",
                                    name=f"osb{i}_{c}")
                nc.vector.tensor_copy(osb[:], ro_ps[c][:])
                osbs.append(osb)
            pending_tail = (i, sacc, osbs, nsl)

        emit_tail(*pending_tail)

    nc.compile()
    return nc


def _get_program():
    if "nc" not in _CACHE:
        _CACHE["nc"] = _build_program()
    return _CACHE["nc"]


def _make_in_maps(mk, qk, mv):
    mk = np.asarray(mk, dtype=np.float32)
    qk = np.asarray(qk, dtype=np.float32)
    mv = np.asarray(mv, dtype=np.float32)
    in_maps = []
    zpad = np.zeros((128 - CK, M), dtype=np.float32)
    for b in range(B):
        mk_b = np.ascontiguousarray(
            np.concatenate([mk[b].reshape(CK, M), zpad], axis=0))
        qk_b = np.ascontiguousarray(
            np.concatenate([qk[b].reshape(CK, N), zpad], axis=0))
        # mkt[p, j*CK + c] = mk[b][c, j*128 + p]
        mkt_b = np.ascontiguousarray(
            mk[b].reshape(CK, MCH, 128).transpose(2, 1, 0).reshape(
                128, MCH * CK))
        # mvt[j, p, c] = mv[b][c, j*128 + p]
        mvt_b = np.ascontiguousarray(
            mv[b].reshape(CV, MCH, 128).transpose(1, 2, 0))
        in_maps.append({"mk": mk_b, "qk": qk_b, "mkt": mkt_b, "mvt": mvt_b})
    return in_maps


def kernel(mk, qk, mv, qv):
    qv = np.asarray(qv, dtype=np.float32)
    nc = _get_program()
    from concourse.bass_utils import run_bass_kernel_spmd

    in_maps = _make_in_maps(mk, qk, mv)
    res = run_bass_kernel_spmd(nc, in_maps, list(range(N_CORES)))
    mem = np.stack([res.results[b]["mem"] for b in range(B)], axis=0)
    mem = mem.reshape(B, CV, H, W)
    return np.concatenate([mem, qv], axis=1)



# revision 8
# speedup vs baseline: 1.3494x; 1.3494x over previous
"""MemoryReader kernel for Trainium2, data-parallel over batch across 8 cores.

Per batch element b (one NeuronCore each):
    mkf = mk[b] as [CK=64, M=4096], qkf = qk[b] as [CK, N=4096]
    aff[m, n] = (2 * mkf.T @ qkf - |mkf[:,m]|^2) / sqrt(CK)
    P = softmax over m
    mem[c, n]  = sum_m mv[b][c, m] * P[m, n]
    out[b] = concat([mem, qv[b]], channel axis)

Device kernel layout (per core):
    - All matmul operands are bf16 (host-cast); PSUM accumulation is fp32.
      bf16 halves LDWEIGHTS/rhs SBUF traffic and input DMA vs fp32r with
      identical 1 cycle/row PE throughput; quantization error ~1.4e-3
      (measured vs fp32 reference) against a 2e-2 budget.
    - Per m-chunk j: QK^T matmul -> [128, 512] PSUM tile; ScalarE computes
      E = exp(0.25*ab - asq/8) straight out of PSUM with the -asq/8 term as
      a per-partition activation bias (no separate g-scaling pass needed).
    - VectorE accumulates sacc += E per chunk; a ones-vector matmul folds
      the partition axis; reciprocal_approx_fast + DMA-free matmul
      broadcast give 1/s across partitions.
    - Readout matmuls trail the QK/exp stream by RO_DELAY chunks
      (software pipeline), contracting over m in PSUM (4 c-chunks of 128).
    - The softmax tail for super i is emitted in two pieces inside super
      i+1 (sum at j==4, broadcast/scale at j==12) so the in-order PE
      stream never waits on the DVE reciprocal.
    - PSUM: 4 rotating QK tiles (shared with the two tail matmuls) +
      4 readout accumulators = 8 banks.
"""

import sys

import numpy as np
import ml_dtypes

B, CK, CV, H, W = 8, 64, 512, 64, 64
M = H * W          # memory positions per batch element
N = H * W          # query positions
NT = 512           # n-super-tile width (columns per softmax pass)
NSUP = N // NT     # 8 n-super-tiles
MCH = M // 128     # 32 m-chunks
N_CORES = 8
RO_DELAY = 4       # readout trails QK/exp by this many m-chunks
N_WARMUP = 36      # PE pstate warmup matmuls

_CACHE = {}


def _build_program():
    sys.path.insert(0, "/opt/trn_rl_repo")
    from contextlib import ExitStack

    import concourse.tile as tile
    from concourse import bacc, mybir

    dt = mybir.dt
    f32 = dt.float32
    bf16 = dt.bfloat16
    # fp32r: fp32 bits with single-pass relaxed PE accumulation; used only
    # for the tiny softmax-sum / broadcast matmuls so they take the
    # 1 cycle/row path.
    f32r = dt.float32r

    nc = bacc.Bacc("TRN2", target_bir_lowering=False, debug=False,
                   num_devices=N_CORES)

    mkt_d = nc.dram_tensor("mkt", [128, MCH * CK], bf16,
                           kind="ExternalInput").ap()
    mk_d = nc.dram_tensor("mk", [128, M], bf16, kind="ExternalInput").ap()
    qk_d = nc.dram_tensor("qk", [128, N], bf16, kind="ExternalInput").ap()
    mvt_d = nc.dram_tensor("mvt", [MCH, 128, CV], bf16,
                           kind="ExternalInput").ap()
    mem_d = nc.dram_tensor("mem", [CV, N], f32, kind="ExternalOutput").ap()

    with tile.TileContext(nc) as tc, ExitStack() as ctx:
        sing = ctx.enter_context(tc.tile_pool(name="sing", bufs=1))
        e_pool = ctx.enter_context(tc.tile_pool(name="E", bufs=8))
        scratch = ctx.enter_context(tc.tile_pool(name="scratch", bufs=2))
        sacc_pool = ctx.enter_context(tc.tile_pool(name="sacc", bufs=2))
        row_pool = ctx.enter_context(tc.tile_pool(name="row", bufs=2))
        rb_pool = ctx.enter_context(tc.tile_pool(name="rb", bufs=2))
        out_pool = ctx.enter_context(tc.tile_pool(name="out", bufs=2))
        qk_ps_pool = ctx.enter_context(
            tc.tile_pool(name="qkps", bufs=4, space="PSUM"))
        ro_ps_pool = ctx.enter_context(
            tc.tile_pool(name="rops", bufs=1, space="PSUM"))

        # PE warmup: the PE activity monitor starts throttled and needs a
        # few us of sustained matmul activity to unthrottle. Burn dummy
        # matmuls while the input DMAs stream.
        warm_sb = sing.tile([128, 128], bf16)
        nc.vector.memset(warm_sb[:], 1.0)
        warm_ps = qk_ps_pool.tile([128, NT], f32, tag="qk_ps", name="warm_ps")
        for w in range(N_WARMUP):
            nc.tensor.matmul(warm_ps[:, 0:128], lhsT=warm_sb[:],
                             rhs=warm_sb[:], start=True, stop=True)

        # Resident inputs. mk/qk are zero-padded from CK=64 to K=128
        # contraction rows (K=64 matmuls leave the PE activity monitor
        # throttled). DMAs are batched (descriptor generation is cheap,
        # dma_start issue on the sync engine costs ~0.8us each) and
        # ordered so the tensors gating the first chunks arrive first:
        # mkt (for the asq bias), mk, first qk super, then mvt in chunk
        # order for the super-0 readout pipeline.
        mkt_sb = sing.tile([128, MCH, CK], bf16)
        mk_sb = sing.tile([128, M], bf16)
        qk_sb = sing.tile([128, N], bf16)
        mvt_sb = sing.tile([128, MCH, CV], bf16)
        nc.sync.dma_start(out=mkt_sb[:], in_=mkt_d[:].rearrange(
            "p (j c) -> p j c", c=CK))
        nc.sync.dma_start(out=mk_sb[:], in_=mk_d[:])
        nc.sync.dma_start(out=qk_sb[:, 0:NT], in_=qk_d[:, 0:NT])
        for grp in range(8):
            js = slice(grp * 4, (grp + 1) * 4)
            nc.sync.dma_start(out=mvt_sb[:, js, :],
                              in_=mvt_d[js].rearrange("j p c -> p j c"))
        nc.sync.dma_start(out=qk_sb[:, NT:N], in_=qk_d[:, NT:N])

        # Ones vectors typed fp32r so the softmax-sum and broadcast
        # matmuls take the single-pass PE path.
        ones_f32 = sing.tile([128, 1], f32)
        nc.vector.memset(ones_f32[:], 1.0)
        ones_col = sing.tile([128, 1], f32r)
        nc.vector.tensor_copy(ones_col[:], ones_f32[:].bitcast(f32r))
        ones_row_f32 = sing.tile([1, 128], f32)
        nc.vector.memset(ones_row_f32[:], 1.0)
        ones_row = sing.tile([1, 128], f32r)
        nc.vector.tensor_copy(ones_row[:], ones_row_f32[:].bitcast(f32r))

        # Per-partition softmax bias: nasq[p, j] = -|mk[:, j*128+p]|^2 / 8,
        # computed from the bf16 mkt so it is exactly consistent with the
        # quantized mk used in the QK matmuls. (tensor_tensor_reduce
        # crashes on HW via this toolchain; Square -> free-axis reduce.)
        nasq = sing.tile([128, MCH], f32)
        for piece in range(4):
            js = slice(piece * 8, (piece + 1) * 8)
            sqp = scratch.tile([128, 8, CK], f32, tag="sqp",
                               name=f"sqp{piece}")
            nc.scalar.activation(sqp[:], mkt_sb[:, js, :],
                                 mybir.ActivationFunctionType.Square)
            nc.vector.tensor_reduce(nasq[:, js], sqp[:],
                                    axis=mybir.AxisListType.X,
                                    op=mybir.AluOpType.add)
        nc.scalar.mul(nasq[:], nasq[:], -0.125)

        Exp = mybir.ActivationFunctionType.Exp

        state = {}

        def emit_tail_sum(i):
            # Softmax denominator for super i: fold sacc's partition axis
            # with a ones matmul, then fast reciprocal on DVE.
            tsacc = state[("sacc", i)]
            s_ps = qk_ps_pool.tile([1, NT], f32, tag="qk_ps",
                                   name=f"sps{i}")
            nc.tensor.matmul(s_ps[:], lhsT=ones_col[:], rhs=tsacc[:],
                             start=True, stop=True)
            s_row = row_pool.tile([1, NT], f32, tag="srow", name=f"srow{i}")
            nc.vector.reciprocal_approx_fast(s_row[:], s_ps[:])
            # walrus requires producers of fp32r matmul operands to be
            # typed fp32r; bridge with a cheap ACT copy.
            s_rowr = row_pool.tile([1, NT], f32r, tag="srowr",
                                   name=f"srowr{i}")
            nc.scalar.copy(s_rowr[:], s_row[:])
            state[("srow", i)] = s_rowr

        def emit_tail_scale(i):
            # Broadcast 1/s across partitions via a rank-1 matmul, scale
            # the evacuated readout, and DMA the super out.
            s_row = state.pop(("srow", i))
            tosb = state.pop(("osb", i))
            rb_ps = qk_ps_pool.tile([128, NT], f32, tag="qk_ps",
                                    name=f"rbps{i}")
            nc.tensor.matmul(rb_ps[:], lhsT=ones_row[:], rhs=s_row[:],
                             start=True, stop=True)
            rb = rb_pool.tile([128, NT], f32, tag="rb", name=f"rb{i}")
            nc.scalar.copy(rb[:], rb_ps[:])
            nsl = slice(i * NT, (i + 1) * NT)
            for c in range(4):
                nc.vector.tensor_mul(tosb[:, c, :], tosb[:, c, :], rb[:])
            nc.sync.dma_start(
                out=mem_d[:, nsl].rearrange("(c p) n -> p c n", p=128),
                in_=tosb[:])

        def emit_front(i, j):
            # QK matmul, exp (with -asq/8 bias), denominator accumulation
            # for chunk j of super i.
            nsl = slice(i * NT, (i + 1) * NT)
            qk_ps = qk_ps_pool.tile([128, NT], f32, tag="qk_ps",
                                    name=f"qkps{i}_{j}")
            nc.tensor.matmul(qk_ps[:],
                             lhsT=mk_sb[:, j * 128:(j + 1) * 128],
                             rhs=qk_sb[:, nsl], start=True, stop=True)
            e = e_pool.tile([128, NT], bf16, tag="E", name=f"e{i}_{j}")
            with nc.allow_low_precision(reason="bf16 E, fp32 accumulate"):
                nc.scalar.activation(e[:], qk_ps[:], Exp,
                                     bias=nasq[:, j:j + 1], scale=0.25)
            if j == 0:
                sacc = sacc_pool.tile([128, NT], f32r, tag="sacc",
                                      name=f"sacc{i}")
                state[("sacc", i)] = sacc
                state[("e", (i, j))] = e
                with nc.allow_low_precision(reason="bf16 in, fp32r out"):
                    nc.vector.tensor_copy(sacc[:], e[:])
            else:
                sacc = state[("sacc", i)]
                state[("e", (i, j))] = e
                with nc.allow_low_precision(reason="bf16 in, fp32r out"):
                    nc.vector.tensor_add(sacc[:], e[:], sacc[:])

        def emit_ro(i, j):
            # Readout matmuls for chunk j of super i; on the last chunk,
            # evacuate the accumulators (emitted before the next sacc op
            # so the DVE drains them promptly for the bank handoff).
            e = state.pop(("e", (i, j)))
            if j == 0:
                state[("rops", i)] = [
                    ro_ps_pool.tile([128, NT], f32, tag=f"ro{c}",
                                    name=f"ro{c}_{i}") for c in range(4)]
            ro_ps = state[("rops", i)]
            for c in range(4):
                nc.tensor.matmul(
                    ro_ps[c][:],
                    lhsT=mvt_sb[:, j, c * 128:(c + 1) * 128],
                    rhs=e[:], start=(j == 0), stop=(j == MCH - 1))
            if j == MCH - 1:
                ro_ps = state.pop(("rops", i))
                osb = out_pool.tile([128, 4, NT], f32, tag="osb",
                                    name=f"osb{i}")
                state[("osb", i)] = osb
                for c in range(4):
                    nc.vector.tensor_copy(osb[:, c, :], ro_ps[c][:])

        # Main software pipeline: QK/exp/sacc lead, readout trails by
        # RO_DELAY chunks, softmax tails of super i-1 are emitted inside
        # super i at j==4 and j==12.
        TOTAL = NSUP * MCH
        for g in range(TOTAL + RO_DELAY):
            if g < TOTAL:
                i, j = divmod(g, MCH)
                if i > 0 and j == 4:
                    emit_tail_sum(i - 1)
                if i > 0 and j == 12:
                    emit_tail_scale(i - 1)
                emit_front(i, j)
            r = g - RO_DELAY
            if r >= 0:
                emit_ro(*divmod(r, MCH))

        emit_tail_sum(NSUP - 1)
        emit_tail_scale(NSUP - 1)

    nc.compile()
    return nc


def _get_program():
    if "nc" not in _CACHE:
        _CACHE["nc"] = _build_program()
    return _CACHE["nc"]


def _make_in_maps(mk, qk, mv):
    bf = ml_dtypes.bfloat16
    mk = np.asarray(mk, dtype=np.float32)
    qk = np.asarray(qk, dtype=np.float32)
    mv = np.asarray(mv, dtype=np.float32)
    in_maps = []
    zpad = np.zeros((128 - CK, M), dtype=bf)
    for b in range(B):
        mk_b = np.ascontiguousarray(
            np.concatenate([mk[b].reshape(CK, M).astype(bf), zpad], axis=0))
        qk_b = np.ascontiguousarray(
            np.concatenate([qk[b].reshape(CK, N).astype(bf), zpad], axis=0))
        # mkt[p, j*CK + c] = mk[b][c, j*128 + p]
        mkt_b = np.ascontiguousarray(
            mk[b].reshape(CK, MCH, 128).transpose(2, 1, 0).reshape(
                128, MCH * CK).astype(bf))
        # mvt[j, p, c] = mv[b][c, j*128 + p]
        mvt_b = np.ascontiguousarray(
            mv[b].reshape(CV, MCH, 128).transpose(1, 2, 0).astype(bf))
        in_maps.append({"mk": mk_b, "qk": qk_b, "mkt": mkt_b, "mvt": mvt_b})
    return in_maps


def kernel(mk, qk, mv, qv):
    qv = np.asarray(qv, dtype=np.float32)
    nc = _get_program()
    from concourse.bass_utils import run_bass_kernel_spmd

    in_maps = _make_in_maps(mk, qk, mv)
    res = run_bass_kernel_spmd(nc, in_maps, list(range(N_CORES)))
    mem = np.stack([res.results[b]["mem"] for b in range(B)], axis=0)
    mem = mem.reshape(B, CV, H, W)
    return np.concatenate([mem, qv], axis=1)


# revision 11
# speedup vs baseline: 1.3582x; 1.0065x over previous
"""MemoryReader kernel for Trainium2, data-parallel over batch across 8 cores.

Per batch element b (one NeuronCore each):
    mkf = mk[b] as [CK=64, M=4096], qkf = qk[b] as [CK, N=4096]
    aff[m, n] = (2 * mkf.T @ qkf - |mkf[:,m]|^2) / sqrt(CK)
    P = softmax over m
    mem[c, n]  = sum_m mv[b][c, m] * P[m, n]
    out[b] = concat([mem, qv[b]], channel axis)

Device kernel layout (per core):
    - All matmul operands are bf16 (host-cast); PSUM accumulation is fp32.
      bf16 halves LDWEIGHTS/rhs SBUF traffic and input DMA vs fp32r with
      identical 1 cycle/row PE throughput; quantization error ~1.4e-3
      (measured vs fp32 reference) against a 2e-2 budget.
    - Per m-chunk j: QK^T matmul -> [128, 512] PSUM tile; ScalarE computes
      E = exp(0.25*ab - asq/8) straight out of PSUM with the -asq/8 term as
      a per-partition activation bias (no separate g-scaling pass needed).
    - VectorE accumulates sacc += E per chunk; a ones-vector matmul folds
      the partition axis; reciprocal_approx_fast + DMA-free matmul
      broadcast give 1/s across partitions.
    - Readout matmuls trail the QK/exp stream by RO_DELAY chunks
      (software pipeline), contracting over m in PSUM (4 c-chunks of 128).
    - The softmax tail for super i is emitted in two pieces inside super
      i+1 (sum at j==4, broadcast/scale at j==12) so the in-order PE
      stream never waits on the DVE reciprocal.
    - PSUM: 4 rotating QK tiles (shared with the two tail matmuls) +
      4 readout accumulators = 8 banks.
"""

import sys

import numpy as np
import ml_dtypes

B, CK, CV, H, W = 8, 64, 512, 64, 64
M = H * W          # memory positions per batch element
N = H * W          # query positions
NT = 512           # n-super-tile width (columns per softmax pass)
NSUP = N // NT     # 8 n-super-tiles
MCH = M // 128     # 32 m-chunks
N_CORES = 8
RO_DELAY = 4       # readout trails QK/exp by this many m-chunks
N_WARMUP = 24      # PE pstate warmup matmuls

_CACHE = {}


def _build_program():
    sys.path.insert(0, "/opt/trn_rl_repo")
    from contextlib import ExitStack

    import concourse.tile as tile
    from concourse import bacc, mybir

    dt = mybir.dt
    f32 = dt.float32
    bf16 = dt.bfloat16
    # fp32r: fp32 bits with single-pass relaxed PE accumulation; used only
    # for the tiny softmax-sum / broadcast matmuls so they take the
    # 1 cycle/row path.
    f32r = dt.float32r

    nc = bacc.Bacc("TRN2", target_bir_lowering=False, debug=False,
                   num_devices=N_CORES)

    mkt_d = nc.dram_tensor("mkt", [128, MCH * CK], bf16,
                           kind="ExternalInput").ap()
    mk_d = nc.dram_tensor("mk", [128, M], bf16, kind="ExternalInput").ap()
    qk_d = nc.dram_tensor("qk", [128, N], bf16, kind="ExternalInput").ap()
    mvt_d = nc.dram_tensor("mvt", [MCH, 128, CV], bf16,
                           kind="ExternalInput").ap()
    mem_d = nc.dram_tensor("mem", [CV, N], f32, kind="ExternalOutput").ap()

    with tile.TileContext(nc) as tc, ExitStack() as ctx:
        sing = ctx.enter_context(tc.tile_pool(name="sing", bufs=1))
        e_pool = ctx.enter_context(tc.tile_pool(name="E", bufs=8))
        scratch = ctx.enter_context(tc.tile_pool(name="scratch", bufs=2))
        sacc_pool = ctx.enter_context(tc.tile_pool(name="sacc", bufs=2))
        row_pool = ctx.enter_context(tc.tile_pool(name="row", bufs=2))
        rb_pool = ctx.enter_context(tc.tile_pool(name="rb", bufs=2))
        out_pool = ctx.enter_context(tc.tile_pool(name="out", bufs=2))
        qk_ps_pool = ctx.enter_context(
            tc.tile_pool(name="qkps", bufs=4, space="PSUM"))
        ro_ps_pool = ctx.enter_context(
            tc.tile_pool(name="rops", bufs=1, space="PSUM"))

        # PE warmup: the PE activity monitor starts throttled and needs a
        # few us of sustained matmul activity to unthrottle. Burn dummy
        # matmuls while the input DMAs stream.
        warm_sb = sing.tile([128, 128], bf16)
        nc.vector.memset(warm_sb[:], 1.0)
        warm_ps = qk_ps_pool.tile([128, NT], f32, tag="qk_ps", name="warm_ps")
        for w in range(N_WARMUP):
            nc.tensor.matmul(warm_ps[:, 0:128], lhsT=warm_sb[:],
                             rhs=warm_sb[:], start=True, stop=True)

        # Resident inputs. mk/qk are zero-padded from CK=64 to K=128
        # contraction rows (K=64 matmuls leave the PE activity monitor
        # throttled). DMAs are batched (descriptor generation is cheap,
        # dma_start issue on the sync engine costs ~0.8us each) and
        # ordered so the tensors gating the first chunks arrive first:
        # mkt (for the asq bias), mk, first qk super, then mvt in chunk
        # order for the super-0 readout pipeline.
        mkt_sb = sing.tile([128, MCH, CK], bf16)
        mk_sb = sing.tile([128, M], bf16)
        qk_sb = sing.tile([128, N], bf16)
        mvt_sb = sing.tile([128, MCH, CV], bf16)
        nc.sync.dma_start(out=qk_sb[:, 0:NT], in_=qk_d[:, 0:NT])
        nc.sync.dma_start(out=mkt_sb[:], in_=mkt_d[:].rearrange(
            "p (j c) -> p j c", c=CK))
        nc.sync.dma_start(out=mk_sb[:, 0:1024], in_=mk_d[:, 0:1024])
        nc.sync.dma_start(out=mk_sb[:, 1024:M], in_=mk_d[:, 1024:M])
        for grp in range(8):
            js = slice(grp * 4, (grp + 1) * 4)
            nc.sync.dma_start(out=mvt_sb[:, js, :],
                              in_=mvt_d[js].rearrange("j p c -> p j c"))
        nc.sync.dma_start(out=qk_sb[:, NT:N], in_=qk_d[:, NT:N])

        # Ones vectors typed fp32r so the softmax-sum and broadcast
        # matmuls take the single-pass PE path.
        ones_f32 = sing.tile([128, 1], f32)
        nc.vector.memset(ones_f32[:], 1.0)
        ones_col = sing.tile([128, 1], f32r)
        nc.vector.tensor_copy(ones_col[:], ones_f32[:].bitcast(f32r))
        ones_row_f32 = sing.tile([1, 128], f32)
        nc.vector.memset(ones_row_f32[:], 1.0)
        ones_row = sing.tile([1, 128], f32r)
        nc.vector.tensor_copy(ones_row[:], ones_row_f32[:].bitcast(f32r))

        # Per-partition softmax bias: nasq[p, j] = -|mk[:, j*128+p]|^2 / 8,
        # computed from the bf16 mkt so it is exactly consistent with the
        # quantized mk used in the QK matmuls. (tensor_tensor_reduce
        # crashes on HW via this toolchain; Square -> free-axis reduce.)
        nasq = sing.tile([128, MCH], f32)
        for piece in range(4):
            js = slice(piece * 8, (piece + 1) * 8)
            sqp = scratch.tile([128, 8, CK], f32, tag="sqp",
                               name=f"sqp{piece}")
            nc.scalar.activation(sqp[:], mkt_sb[:, js, :],
                                 mybir.ActivationFunctionType.Square)
            nc.vector.tensor_reduce(nasq[:, js], sqp[:],
                                    axis=mybir.AxisListType.X,
                                    op=mybir.AluOpType.add)
        nc.scalar.mul(nasq[:], nasq[:], -0.125)

        Exp = mybir.ActivationFunctionType.Exp

        state = {}

        def emit_tail_sum(i):
            # Softmax denominator for super i: fold sacc's partition axis
            # with a ones matmul, then fast reciprocal on DVE.
            tsacc = state[("sacc", i)]
            s_ps = qk_ps_pool.tile([1, NT], f32, tag="qk_ps",
                                   name=f"sps{i}")
            nc.tensor.matmul(s_ps[:], lhsT=ones_col[:], rhs=tsacc[:],
                             start=True, stop=True)
            s_row = row_pool.tile([1, NT], f32, tag="srow", name=f"srow{i}")
            nc.vector.reciprocal_approx_fast(s_row[:], s_ps[:])
            # walrus requires producers of fp32r matmul operands to be
            # typed fp32r; bridge with a cheap ACT copy.
            s_rowr = row_pool.tile([1, NT], f32r, tag="srowr",
                                   name=f"srowr{i}")
            nc.scalar.copy(s_rowr[:], s_row[:])
            state[("srow", i)] = s_rowr

        def emit_tail_scale(i):
            # Broadcast 1/s across partitions via a rank-1 matmul, scale
            # the evacuated readout, and DMA the super out.
            s_row = state.pop(("srow", i))
            tosb = state.pop(("osb", i))
            rb_ps = qk_ps_pool.tile([128, NT], f32, tag="qk_ps",
                                    name=f"rbps{i}")
            nc.tensor.matmul(rb_ps[:], lhsT=ones_row[:], rhs=s_row[:],
                             start=True, stop=True)
            rb = rb_pool.tile([128, NT], f32, tag="rb", name=f"rb{i}")
            nc.scalar.copy(rb[:], rb_ps[:])
            nsl = slice(i * NT, (i + 1) * NT)
            mem_v = mem_d[:, nsl].rearrange("(c p) n -> p c n", p=128)
            for c in range(4):
                nc.vector.tensor_mul(tosb[:, c, :], tosb[:, c, :], rb[:])
                nc.sync.dma_start(out=mem_v[:, c, :], in_=tosb[:, c, :])

        def emit_front(i, j):
            # QK matmul, exp (with -asq/8 bias), denominator accumulation
            # for chunk j of super i.
            nsl = slice(i * NT, (i + 1) * NT)
            qk_ps = qk_ps_pool.tile([128, NT], f32, tag="qk_ps",
                                    name=f"qkps{i}_{j}")
            nc.tensor.matmul(qk_ps[:],
                             lhsT=mk_sb[:, j * 128:(j + 1) * 128],
                             rhs=qk_sb[:, nsl], start=True, stop=True)
            e = e_pool.tile([128, NT], bf16, tag="E", name=f"e{i}_{j}")
            with nc.allow_low_precision(reason="bf16 E, fp32 accumulate"):
                nc.scalar.activation(e[:], qk_ps[:], Exp,
                                     bias=nasq[:, j:j + 1], scale=0.25)
            if j == 0:
                sacc = sacc_pool.tile([128, NT], f32r, tag="sacc",
                                      name=f"sacc{i}")
                state[("sacc", i)] = sacc
                state[("e", (i, j))] = e
                with nc.allow_low_precision(reason="bf16 in, fp32r out"):
                    nc.vector.tensor_copy(sacc[:], e[:])
            else:
                sacc = state[("sacc", i)]
                state[("e", (i, j))] = e
                with nc.allow_low_precision(reason="bf16 in, fp32r out"):
                    nc.vector.tensor_add(sacc[:], e[:], sacc[:])

        def emit_ro(i, j):
            # Readout matmuls for chunk j of super i; on the last chunk,
            # evacuate the accumulators (emitted before the next sacc op
            # so the DVE drains them promptly for the bank handoff).
            e = state.pop(("e", (i, j)))
            if j == 0:
                state[("rops", i)] = [
                    ro_ps_pool.tile([128, NT], f32, tag=f"ro{c}",
                                    name=f"ro{c}_{i}") for c in range(4)]
            ro_ps = state[("rops", i)]
            for c in range(4):
                nc.tensor.matmul(
                    ro_ps[c][:],
                    lhsT=mvt_sb[:, j, c * 128:(c + 1) * 128],
                    rhs=e[:], start=(j == 0), stop=(j == MCH - 1))
            if j == MCH - 1:
                ro_ps = state.pop(("rops", i))
                osb = out_pool.tile([128, 4, NT], f32, tag="osb",
                                    name=f"osb{i}")
                state[("osb", i)] = osb
                for c in range(4):
                    nc.vector.tensor_copy(osb[:, c, :], ro_ps[c][:])

        # Main software pipeline: QK/exp/sacc lead, readout trails by
        # RO_DELAY chunks, softmax tails of super i-1 are emitted inside
        # super i at j==4 and j==12.
        TOTAL = NSUP * MCH
        for g in range(TOTAL + RO_DELAY):
            if g < TOTAL:
                i, j = divmod(g, MCH)
                if i > 0 and j == 4:
                    emit_tail_sum(i - 1)
                if i > 0 and j == 12:
                    emit_tail_scale(i - 1)
                emit_front(i, j)
            r = g - RO_DELAY
            if r >= 0:
                emit_ro(*divmod(r, MCH))

        emit_tail_sum(NSUP - 1)
        emit_tail_scale(NSUP - 1)

    nc.compile()
    return nc


def _get_program():
    if "nc" not in _CACHE:
        _CACHE["nc"] = _build_program()
    return _CACHE["nc"]


def _make_in_maps(mk, qk, mv):
    bf = ml_dtypes.bfloat16
    mk = np.asarray(mk, dtype=np.float32)
    qk = np.asarray(qk, dtype=np.float32)
    mv = np.asarray(mv, dtype=np.float32)
    in_maps = []
    zpad = np.zeros((128 - CK, M), dtype=bf)
    for b in range(B):
        mk_b = np.ascontiguousarray(
            np.concatenate([mk[b].reshape(CK, M).astype(bf), zpad], axis=0))
        qk_b = np.ascontiguousarray(
            np.concatenate([qk[b].reshape(CK, N).astype(bf), zpad], axis=0))
        # mkt[p, j*CK + c] = mk[b][c, j*128 + p]
        mkt_b = np.ascontiguousarray(
            mk[b].reshape(CK, MCH, 128).transpose(2, 1, 0).reshape(
                128, MCH * CK).astype(bf))
        # mvt[j, p, c] = mv[b][c, j*128 + p]
        mvt_b = np.ascontiguousarray(
            mv[b].reshape(CV, MCH, 128).transpose(1, 2, 0).astype(bf))
        in_maps.append({"mk": mk_b, "qk": qk_b, "mkt": mkt_b, "mvt": mvt_b})
    return in_maps


def kernel(mk, qk, mv, qv):
    qv = np.asarray(qv, dtype=np.float32)
    nc = _get_program()
    from concourse.bass_utils import run_bass_kernel_spmd

    in_maps = _make_in_maps(mk, qk, mv)
    res = run_bass_kernel_spmd(nc, in_maps, list(range(N_CORES)))
    mem = np.stack([res.results[b]["mem"] for b in range(B)], axis=0)
    mem = mem.reshape(B, CV, H, W)
    return np.concatenate([mem, qv], axis=1)
